# revision 14
# baseline (speedup 1.0000x reference)
"""GRU free-run greedy decoder on 8 Trainium2 NeuronCores (data parallel).

Problem: 2-layer GRU (H=512) + fc(V=256) greedy decode, T=64 steps,
B=1024 batch, latent LAT=256 concatenated with previous one-hot as input.

Sharding: pure data parallel. Each of the 8 cores handles 128 batch rows
(= exactly the 128 SBUF partitions). GRU + fc weights are replicated.
The whole recurrence runs on-chip; only the final [128, T, V] one-hot
stream is DMA'd out (as fp16, exact for one-hots).

Matmul mapping: out[batch, outdim] = lhsT.T @ rhs with
  lhsT (stationary) = activation^T chunk [K=128, 128 batch]
  rhs  (moving)     = weight^T chunk     [K=128, <=512 outdim]

Precision: h-dependent matmuls (hh0 / ih1 / hh1 / fc) run as 3-term fp16
split products accumulated in fp32 PSUM:
    h @ W ~= a@c + (a*2^-12)@d_s + b@c,
    a=f16(h), b=f16(h-a), c=f16(W), d_s=f16((W-c)*2^12)
The scaled d_s pair keeps the W residual at full fp16 precision (W
captured to ~2^-23); b sits partly in fp16 subnormal range, which the PE
honors exactly (verified by HW probe) and is quantum-2^-24-absolute, so
h is captured to ~2^-24 too. The numpy-emulated trajectory of this
scheme matches the fp64 reference argmax for every token, while all
1/2-term variants flip hundreds of tokens. Cost: 12 fp16 chunk-streams
per 512-K matmul vs fp32's 16 cycle-equivalents, i.e. 25% less PE time
on the dominant matmuls plus cheaper transposes/copies.

One-hot embedding stays an EXACT 2-pass fp16 scheme; Lc / layer-1 / fc
biases are added on the Vector engine (frees all PE bias-seed matmuls).
"""

import sys
import numpy as np

sys.path.insert(0, "/opt/trn_rl_repo")

P = 128          # partitions == per-core batch
H = 512          # hidden
V = 256          # vocab
LAT = 256        # latent dim
G3 = 3 * H       # 1536 gate width
T_FULL = 64
N_CORES = 8

_CACHE = {}


def build_program(T=T_FULL):
    """Build + compile the Bass program. Returns the compiled Bacc object."""
    import concourse.bass as bass
    import concourse.tile as tile
    from concourse import bacc, mybir
    from concourse.masks import make_identity

    f32 = mybir.dt.float32
    f16 = mybir.dt.float16
    bf16 = mybir.dt.bfloat16
    AF = mybir.ActivationFunctionType
    OP = mybir.AluOpType
    ts = bass.ts

    nc = bacc.Bacc(
        "TRN2", target_bir_lowering=False, debug=False,
        enable_asserts=False, num_devices=N_CORES,
    )

    # ---- DRAM I/O (weights pre-laid-out [P, kc, ...] for single DMAs;
    # Lc = latent @ W_lat + biases is a one-time input-prep product,
    # precomputed on host like the weight transposes/splits) ----
    lcrz_d = nc.dram_tensor("lcrz", [P, 2 * H], f32, kind="ExternalInput").ap()
    nb0_d = nc.dram_tensor("nb0", [P, 2 * H], f32, kind="ExternalInput").ap()
    wembh_d = nc.dram_tensor("wembh", [P, 2, G3], f16, kind="ExternalInput").ap()
    wembl_d = nc.dram_tensor("wembl", [P, 2, G3], f16, kind="ExternalInput").ap()
    whh0c_d = nc.dram_tensor("whh0c", [P, 4, G3], f16, kind="ExternalInput").ap()
    whh0d_d = nc.dram_tensor("whh0d", [P, 4, G3], f16, kind="ExternalInput").ap()
    wih1c_d = nc.dram_tensor("wih1c", [P, 4, G3], f16, kind="ExternalInput").ap()
    wih1d_d = nc.dram_tensor("wih1d", [P, 4, G3], f16, kind="ExternalInput").ap()
    whh1c_d = nc.dram_tensor("whh1c", [P, 4, G3], f16, kind="ExternalInput").ap()
    whh1d_d = nc.dram_tensor("whh1d", [P, 4, G3], f16, kind="ExternalInput").ap()
    wfcc_d = nc.dram_tensor("wfcc", [P, 4, V], f16, kind="ExternalInput").ap()
    wfcd_d = nc.dram_tensor("wfcd", [P, 4, V], f16, kind="ExternalInput").ap()
    b1rz_d = nc.dram_tensor("b1rz", [P, 2 * H], f32, kind="ExternalInput").ap()
    b1nb_d = nc.dram_tensor("b1nb", [P, 2 * H], f32, kind="ExternalInput").ap()
    bfc_d = nc.dram_tensor("bfc", [P, V], f32, kind="ExternalInput").ap()
    out_d = nc.dram_tensor("out", [P, T, V], f16, kind="ExternalOutput").ap()

    from contextlib import ExitStack
    with tile.TileContext(nc) as tc, ExitStack() as ctx:
        wt = ctx.enter_context(tc.tile_pool(name="wt", bufs=1))
        st = ctx.enter_context(tc.tile_pool(name="st", bufs=1))
        wk = ctx.enter_context(tc.tile_pool(name="wk", bufs=2))
        # PSUM (8 banks): rz 2x[P,1024]f32 double-buffered (4 banks),
        # ihn [P,1024]f32 (2), fc [P,256]f32 (1), f16 transpose scratch (1).
        ps = ctx.enter_context(tc.tile_pool(name="ps", bufs=2, space="PSUM"))
        ps1 = ctx.enter_context(tc.tile_pool(name="ps1", bufs=1, space="PSUM"))

        # ---- persistent weights/biases in SBUF ----
        whh0c = wt.tile([P, 4, G3], f16, tag="whh0c")
        whh0d = wt.tile([P, 4, G3], f16, tag="whh0d")
        wih1c = wt.tile([P, 4, G3], f16, tag="wih1c")
        wih1d = wt.tile([P, 4, G3], f16, tag="wih1d")
        whh1c = wt.tile([P, 4, G3], f16, tag="whh1c")
        whh1d = wt.tile([P, 4, G3], f16, tag="whh1d")
        wembh = wt.tile([P, 2, G3], f16, tag="wembh")
        wembl = wt.tile([P, 2, G3], f16, tag="wembl")
        wfcc = wt.tile([P, 4, V], f16, tag="wfcc")
        wfcd = wt.tile([P, 4, V], f16, tag="wfcd")
        lcrz = wt.tile([P, 2 * H], f32, tag="lcrz")
        nb0 = wt.tile([P, 2 * H], f32, tag="nb0")
        b1rz = wt.tile([P, 2 * H], f32, tag="b1rz")
        b1nb = wt.tile([P, 2 * H], f32, tag="b1nb")
        bfc = wt.tile([P, V], f32, tag="bfc")
        # One contiguous DMA per tensor, ordered by first use; step-0 needs
        # lcrz/nb0/wih1/wfc (sync queue), step-1 weights ride the second
        # hwdge queue (Activation) in parallel.
        nc.sync.dma_start(lcrz[:], lcrz_d[:])
        nc.sync.dma_start(nb0[:], nb0_d[:])
        nc.sync.dma_start(b1rz[:], b1rz_d[:])
        nc.sync.dma_start(b1nb[:], b1nb_d[:])
        nc.sync.dma_start(bfc[:], bfc_d[:])
        def flat(tile3):
            return tile3[:, :, :].rearrange("p a b -> p (a b)")
        nc.sync.dma_start(flat(wih1c), flat(wih1c_d))
        nc.sync.dma_start(flat(wih1d), flat(wih1d_d))
        nc.sync.dma_start(flat(wfcc), flat(wfcc_d))
        nc.sync.dma_start(flat(wfcd), flat(wfcd_d))
        nc.scalar.dma_start(flat(whh0c), flat(whh0c_d))
        nc.scalar.dma_start(flat(whh0d), flat(whh0d_d))
        nc.scalar.dma_start(flat(whh1c), flat(whh1c_d))
        nc.scalar.dma_start(flat(whh1d), flat(whh1d_d))
        nc.scalar.dma_start(flat(wembh), flat(wembh_d))
        nc.scalar.dma_start(flat(wembl), flat(wembl_d))

        zer = wt.tile([P, H], bf16, tag="zer")
        nc.gpsimd.memset(zer[:], 0.0)
        identb = wt.tile([P, P], bf16, tag="identb")
        make_identity(nc, identb[:])
        identf = wt.tile([P, P], f16, tag="identf")
        make_identity(nc, identf[:])

        # ---- persistent state ----
        h0 = st.tile([P, H], f32, tag="h0")
        h1 = st.tile([P, H], f32, tag="h1")
        # (Lc and the combined layer-0 n-bias arrive precomputed via DMA)
        h0Ta = st.tile([P, 4, P], f16, tag="h0Ta")
        h0Tas = st.tile([P, 4, P], f16, tag="h0Tas")
        h0Tb = st.tile([P, 4, P], f16, tag="h0Tb")
        h1Ta = st.tile([P, 4, P], f16, tag="h1Ta")
        h1Tas = st.tile([P, 4, P], f16, tag="h1Tas")
        h1Tb = st.tile([P, 4, P], f16, tag="h1Tb")
        ohT = st.tile([P, 2, P], f16, tag="ohT")
        for tl in (h0, h1):
            nc.gpsimd.memset(tl[:], 0.0)

        def zero_mm(dest):
            """Write zeros to a [P, n] psum region via bf16 zero-matmuls."""
            n = dest.shape[-1]
            for ci in range(0, n, 512):
                w = min(512, n - ci)
                nc.tensor.matmul(dest[:, ci:ci + w], identb[:], zer[:, 0:w],
                                 start=True, stop=True)

        def split_h(a, b, ha, has, hb, trsp, cols, tag):
            """Transpose the f16 hi/lo split (a, b from gru_gates) into
            sbuf [P,4,P] f16, plus a*2^-12 (pairs with the *2^12-scaled W
            residual; exponent shift, exact). trsp: [P,1024] f16 psum."""
            ab, bb = cols
            for kc in range(4):
                nc.tensor.transpose(trsp[:, ab + kc * P:ab + (kc + 1) * P],
                                    a[:, ts(kc, P)], identf[:])
            hav = ha[:, :, :].rearrange("p a b -> p (a b)")
            nc.scalar.copy(hav, trsp[:, ab:ab + 512])
            nc.scalar.mul(has[:, :, :].rearrange("p a b -> p (a b)"),
                          trsp[:, ab:ab + 512], 2.0 ** -12)
            for kc in range(4):
                nc.tensor.transpose(trsp[:, bb + kc * P:bb + (kc + 1) * P],
                                    b[:, ts(kc, P)], identf[:])
            nc.scalar.copy(hb[:, :, :].rearrange("p a b -> p (a b)"),
                           trsp[:, bb:bb + 512])

        def big_mm_rz(grz, ha, has, hb, wc, wd, first_rz, last_rz,
                      js=(0, 1)):
            """rz part of a 3-term f16 split matmul into [P,1024] psum.
            The full r-half (j=0) is emitted before the z-half so the
            r-sigmoid chain can start ~2.5us before z completes."""
            terms = ((ha, wc), (has, wd), (hb, wc))
            for j in js:
                for kc in range(4):
                    for ti, (s, m) in enumerate(terms):
                        fst = first_rz and kc == 0 and ti == 0
                        lst = last_rz and kc == 3 and ti == 2
                        nc.tensor.matmul(grz[:, ts(j, 512)], s[:, kc, :],
                                         m[:, kc, ts(j, 512)],
                                         start=fst, stop=lst)

        def big_mm_n(gn, ha, has, hb, wc, wd, gn_sl):
            """h_n / i_n part of a 3-term f16 split matmul (own group)."""
            terms = ((ha, wc), (has, wd), (hb, wc))
            for kc in range(4):
                for ti, (s, m) in enumerate(terms):
                    nc.tensor.matmul(gn[:, gn_sl], s[:, kc, :],
                                     m[:, kc, 1024:1536],
                                     start=(kc == 0 and ti == 0),
                                     stop=(kc == 3 and ti == 2))

        def gru_gates(grz, gihn, rzbias, nb, h, tag):
            """gates + state update for one layer; h updated in place.
            Latency-ordered: the r chain (rt-add -> sigmoid -> r*hn) starts
            as soon as the r psum half completes; z-side work (u=1-z,
            zh=z*h_old) runs during the tanh; h' = u*n + zh. Also emits the
            f16 hi split a = f16(h') one DVE op early and b = f16(h'-a).
            Returns (a, b) for the transposes."""
            rt = wk.tile([P, H], f32, tag="rt", name=f"rt{tag}")
            nc.vector.tensor_add(rt[:], grz[:, 0:512], rzbias[:, 0:512])
            rr = wk.tile([P, H], f32, tag="rr", name=f"rr{tag}")
            nc.scalar.activation(rr[:], rt[:], AF.Sigmoid)
            hn = wk.tile([P, H], f32, tag="hn", name=f"hn{tag}")
            nc.vector.tensor_add(hn[:], gihn[:, 512:1024], nb[:, 512:1024])
            rhn = wk.tile([P, H], f32, tag="rhn", name=f"rhn{tag}")
            nc.vector.tensor_mul(rhn[:], rr[:], hn[:])
            zt = wk.tile([P, H], f32, tag="zt", name=f"zt{tag}")
            nc.vector.tensor_add(zt[:], grz[:, 512:1024], rzbias[:, 512:1024])
            zz = wk.tile([P, H], f32, tag="zz", name=f"zz{tag}")
            nc.scalar.activation(zz[:], zt[:], AF.Sigmoid)
            inn = wk.tile([P, H], f32, tag="inn", name=f"inn{tag}")
            nc.vector.tensor_add(inn[:], gihn[:, 0:512], nb[:, 0:512])
            npre = wk.tile([P, H], f32, tag="npre", name=f"npre{tag}")
            nc.vector.tensor_add(npre[:], inn[:], rhn[:])
            nn = wk.tile([P, H], f32, tag="nn", name=f"nn{tag}")
            nc.scalar.activation(nn[:], npre[:], AF.Tanh)
            # off-chain while tanh runs: u = 1-z, zh = z*h_old
            uu = wk.tile([P, H], f32, tag="uu", name=f"uu{tag}")
            nc.vector.tensor_scalar(uu[:], zz[:], -1.0, 1.0,
                                    op0=OP.mult, op1=OP.add)
            zh = wk.tile([P, H], f32, tag="zh", name=f"zh{tag}")
            nc.vector.tensor_mul(zh[:], zz[:], h[:])
            un = wk.tile([P, H], f32, tag="un", name=f"un{tag}")
            nc.vector.tensor_mul(un[:], uu[:], nn[:])
            a = wk.tile([P, H], f16, tag="spa", name=f"spa{tag}")
            nc.vector.tensor_add(a[:], un[:], zh[:])
            nc.vector.tensor_add(h[:], un[:], zh[:])
            b = wk.tile([P, H], f16, tag="spb", name=f"spb{tag}")
            nc.vector.tensor_sub(b[:], h[:], a[:])
            return a, b

        # ---- the T decode steps, software-pipelined: step t's hh0/gh1-rz
        # matmuls are emitted before step t-1's argmax tail, so the PE chews
        # on them while DVE finishes t-1. ----
        def argmax_tail(t, trsp, lg):
            """lg+bias -> argmax -> one-hot f16 -> DMA + ohT.
            Fused: one DVE op adds the bias and reduces the row max; a
            second emits the one-hot. Exact fp32 logit ties never occur on
            this trajectory (checked: min top1-top2 gap is 7e-7 >> the
            ~3e-8 kernel error), so is_equal marks exactly one element."""
            lgb = wk.tile([P, V], f32, tag="lgb", name=f"lgb_{t}")
            nc.vector.tensor_add(lgb[:], lg, bfc[:])
            mx = wk.tile([P, 1], f32, tag="mx", name=f"mx_{t}")
            nc.vector.reduce_max(mx[:], lgb[:], axis=mybir.AxisListType.X)
            oh = wk.tile([P, V], f16, tag="oh", name=f"oh_{t}")
            nc.vector.tensor_scalar(oh[:], lgb[:], mx[:, 0:1], None,
                                    op0=OP.is_equal)
            nc.sync.dma_start(out_d[:, t, :], oh[:])
            if trsp is not None:
                for v in range(2):
                    nc.tensor.transpose(trsp[:, v * P:(v + 1) * P],
                                        oh[:, ts(v, P)], identf[:])
                nc.scalar.copy(ohT[:, :, :].rearrange("p a b -> p (a b)"),
                               trsp[:, 0:256])

        # Software pipeline: iteration t emits step t's gates plus step
        # t+1's hh0 matmuls, placed so the in-order PE always has runnable
        # work while the serial DVE/ACT gate chains execute:
        #   gh1-rz(t) bridges the t-1 argmax tail, gh1-n(t) bridges
        #   gates0(t), hh0-rz(t+1) bridges gates1(t).
        prev_lg = None
        g0rz = ps.tile([P, 1024], f32, tag="rz", name="g0rz_0")
        g0ihn = ps1.tile([P, 1024], f32, tag="ihn", name="g0ihn_0")
        zero_mm(g0rz)
        zero_mm(g0ihn[:, 512:1024])
        for t in range(T):
            # -- gh1 rz terms (h1T from t-1); gi1 rz closes the group --
            g1rz = ps.tile([P, 1024], f32, tag="rz", name=f"g1rz_{t}")
            if t > 0:
                big_mm_rz(g1rz, h1Ta, h1Tas, h1Tb, whh1c, whh1d,
                          first_rz=True, last_rz=False)
            # at t=0 gh1 is skipped; gi1 opens the g1rz group instead

            # -- step t-1 tail: argmax -> one-hot -> ohT (DVE/ACT work) --
            if t > 0:
                trsp_oh = ps1.tile([P, 1024], f16, tag="trsp",
                                   name=f"trsp_oh_{t}")
                argmax_tail(t - 1, trsp_oh, prev_lg)

            # -- emb finishes layer0 groups (needs ohT from t-1 tail);
            #    EXACT 2-pass fp16; regions complete r -> i_n -> z --
            if t > 0:
                passes = ((ohT, wembh), (ohT, wembl))
                for j in (0, None, 1):
                    if j is None:
                        for pi, (oh_s, hl) in enumerate(passes):
                            for v in range(2):
                                nc.tensor.matmul(
                                    g0ihn[:, 0:512], oh_s[:, v, :],
                                    hl[:, v, 1024:1536],
                                    start=(pi == 0 and v == 0),
                                    stop=(pi == 1 and v == 1))
                        continue
                    for pi, (oh_s, hl) in enumerate(passes):
                        for v in range(2):
                            nc.tensor.matmul(g0rz[:, ts(j, 512)],
                                             oh_s[:, v, :],
                                             hl[:, v, ts(j, 512)],
                                             start=False,
                                             stop=(pi == 1 and v == 1))
            else:
                zero_mm(g0ihn[:, 0:512])

            # -- layer0 gates -> h0 (in place) --
            a0, b0 = gru_gates(g0rz, g0ihn, lcrz[:], nb0[:], h0,
                               f"0_{t}")

            # -- gh1 h_n: runnable while DVE computes the l0 gates --
            g1ihn = ps1.tile([P, 1024], f32, tag="ihn", name=f"g1ihn_{t}")
            if t > 0:
                big_mm_n(g1ihn, h1Ta, h1Tas, h1Tb, whh1c, whh1d,
                         slice(512, 1024))
            else:
                zero_mm(g1ihn[:, 512:1024])

            # -- h0' split transposes -> h0Ta/h0Tb --
            trsp0 = ps1.tile([P, 1024], f16, tag="trsp", name=f"trsp0_{t}")
            split_h(a0, b0, h0Ta, h0Tas, h0Tb, trsp0, (0, 512), f"0_{t}")

            # -- gi1 (= h0' @ Wih1): r first, then i_n, then z --
            big_mm_rz(g1rz, h0Ta, h0Tas, h0Tb, wih1c, wih1d,
                      first_rz=(t == 0), last_rz=True, js=(0,))
            big_mm_n(g1ihn, h0Ta, h0Tas, h0Tb, wih1c, wih1d, slice(0, 512))
            big_mm_rz(g1rz, h0Ta, h0Tas, h0Tb, wih1c, wih1d,
                      first_rz=(t == 0), last_rz=True, js=(1,))

            # -- hh0-rz(t+1): ready as soon as h0T(t) lands; bridges the
            # gates1(t) serial chain on the in-order PE --
            if t + 1 < T:
                g0rz = ps.tile([P, 1024], f32, tag="rz", name=f"g0rz_{t+1}")
                big_mm_rz(g0rz, h0Ta, h0Tas, h0Tb, whh0c, whh0d,
                          first_rz=True, last_rz=False)

            # -- layer1 gates -> h1 (in place) --
            a1, b1 = gru_gates(g1rz, g1ihn, b1rz[:], b1nb[:], h1, f"1_{t}")

            # -- h1' split transposes -> h1Ta/h1Tb --
            trsp1 = ps1.tile([P, 1024], f16, tag="trsp", name=f"trsp1_{t}")
            split_h(a1, b1, h1Ta, h1Tas, h1Tb, trsp1, (0, 512), f"1_{t}")

            # -- fc logits (3-term f16) -> fc psum --
            fcp = ps1.tile([P, V], f32, tag="fc", name=f"fc_{t}")
            fterms = ((h1Ta, wfcc), (h1Tas, wfcd), (h1Tb, wfcc))
            for kc in range(4):
                for ti, (s, m) in enumerate(fterms):
                    nc.tensor.matmul(fcp[:], s[:, kc, :], m[:, kc, :],
                                     start=(kc == 0 and ti == 0),
                                     stop=(kc == 3 and ti == 2))
            prev_lg = fcp[:]

            # -- hh0-n(t+1): needs the ihn buffer freed by gates1(t)'s
            # iht read; fills the tail while DVE runs argmax(t) --
            if t + 1 < T:
                g0ihn = ps1.tile([P, 1024], f32, tag="ihn",
                                 name=f"g0ihn_{t+1}")
                big_mm_n(g0ihn, h0Ta, h0Tas, h0Tb, whh0c, whh0d,
                         slice(512, 1024))

        argmax_tail(T - 1, None, prev_lg)

    nc.compile()
    return nc


def prep_host_inputs(latent_vec, w_ih0, w_hh0, b_ih0, b_hh0,
                     w_ih_r, w_hh_r, b_ih_r, b_hh_r, w_fc, b_fc):
    """Pure-layout host prep: transposes/reshapes, f16 pair splits, bias
    merge/replicate. Returns per-core in_maps."""
    f4 = np.float32
    f16 = np.float16

    def rep(v):  # replicate a [N] vector across the 128 partitions
        return np.ascontiguousarray(np.broadcast_to(v.astype(f4), (P, v.shape[0])))

    def split_f16(a):  # c = f16(a), d_s = f16((a-c)*2^12); 3-term operands
        c = a.astype(f16)
        d = ((a - c.astype(f4)) * 4096.0).astype(f16)
        return c, d

    def core_layout(a):  # [kc, P, n] -> contiguous [P, kc, n]
        return np.ascontiguousarray(a.transpose(1, 0, 2))

    wembT = np.ascontiguousarray(w_ih0[:, LAT:].T.astype(f4)).reshape(2, P, G3)
    wembh = wembT.astype(f16)
    # residual stored unscaled: fp16 subnormals are honored by the PE and
    # carry 2^-24-absolute quanta, exact enough for the one-hot selection
    wembl = (wembT - wembh.astype(f4)).astype(f16)
    whh0c, whh0d = split_f16(w_hh0.T.astype(f4).reshape(4, P, G3))
    wih1c, wih1d = split_f16(w_ih_r[0].T.astype(f4).reshape(4, P, G3))
    whh1c, whh1d = split_f16(w_hh_r[0].T.astype(f4).reshape(4, P, G3))
    wfcc, wfcd = split_f16(w_fc.T.astype(f4).reshape(4, P, V))

    # Lc = latent @ W_lat^T + (b_ih0 + rz-part of b_hh0): per-batch-row,
    # step-invariant -> precomputed host-side like the weight splits
    blc_v = b_ih0.astype(f4).copy()
    blc_v[:1024] += b_hh0[:1024].astype(f4)
    Lc_full = latent_vec.astype(f4) @ w_ih0[:, :LAT].T.astype(f4) + blc_v

    common = dict(
        wembh=core_layout(wembh), wembl=core_layout(wembl),
        whh0c=core_layout(whh0c), whh0d=core_layout(whh0d),
        wih1c=core_layout(wih1c), wih1d=core_layout(wih1d),
        whh1c=core_layout(whh1c), whh1d=core_layout(whh1d),
        wfcc=core_layout(wfcc), wfcd=core_layout(wfcd),
        b1rz=rep((b_ih_r[0] + b_hh_r[0])[:1024]),
        b1nb=rep(np.concatenate([b_ih_r[0][1024:], b_hh_r[0][1024:]])),
        bfc=rep(b_fc),
    )
    b0hn_r = rep(b_hh0[1024:])
    in_maps = []
    for c in range(N_CORES):
        m = dict(common)
        sl = Lc_full[c * P:(c + 1) * P]
        m["lcrz"] = np.ascontiguousarray(sl[:, 0:1024])
        m["nb0"] = np.ascontiguousarray(
            np.concatenate([sl[:, 1024:1536], b0hn_r], axis=1))
        in_maps.append(m)
    return in_maps


def kernel(**inputs):
    from concourse import bass_utils

    key = ("prog", T_FULL)
    if key not in _CACHE:
        _CACHE[key] = build_program(T_FULL)
    nc = _CACHE[key]

    in_maps = prep_host_inputs(
        np.asarray(inputs["latent_vec"]), np.asarray(inputs["w_ih0"]),
        np.asarray(inputs["w_hh0"]), np.asarray(inputs["b_ih0"]),
        np.asarray(inputs["b_hh0"]), np.asarray(inputs["w_ih_r"]),
        np.asarray(inputs["w_hh_r"]), np.asarray(inputs["b_ih_r"]),
        np.asarray(inputs["b_hh_r"]), np.asarray(inputs["w_fc"]),
        np.asarray(inputs["b_fc"]))

    res = bass_utils.run_bass_kernel_spmd(nc, in_maps, list(range(N_CORES)))
    out = np.concatenate([res.results[c]["out"] for c in range(N_CORES)], axis=0)
    return out.astype(np.float32)


# revision 15
# speedup vs baseline: 1.1887x; 1.1887x over previous
"""GRU free-run greedy decoder on 8 Trainium2 NeuronCores (data parallel).

Problem: 2-layer GRU (H=512) + fc(V=256) greedy decode, T=64 steps,
B=1024 batch, latent LAT=256 concatenated with previous one-hot as input.

Sharding: pure data parallel. Each of the 8 cores handles 128 batch rows
(= exactly the 128 SBUF partitions). GRU + fc weights are replicated.
The whole recurrence runs on-chip; only the final [128, T, V] one-hot
stream is DMA'd out (as fp16, exact for one-hots).

Matmul mapping: out[batch, outdim] = lhsT.T @ rhs with
  lhsT (stationary) = activation^T chunk [K=128, 128 batch]
  rhs  (moving)     = weight^T chunk     [K=128, <=512 outdim]

Precision: h-dependent matmuls (hh0 / ih1 / hh1 / fc) run as 3-term fp16
split products accumulated in fp32 PSUM:
    h @ W ~= a@c + (a*2^-12)@d_s + b@c,
    a=f16(h), b=f16(h-a), c=f16(W), d_s=f16((W-c)*2^12)
The scaled d_s pair keeps the W residual at full fp16 precision (W
captured to ~2^-23); b sits partly in fp16 subnormal range, which the PE
honors exactly (verified by HW probe) and is quantum-2^-24-absolute, so
h is captured to ~2^-24 too. The numpy-emulated trajectory of this
scheme matches the fp64 reference argmax for every token, while all
1/2-term variants flip hundreds of tokens. Cost: 12 fp16 chunk-streams
per 512-K matmul vs fp32's 16 cycle-equivalents, i.e. 25% less PE time
on the dominant matmuls plus cheaper transposes/copies.

One-hot embedding stays an EXACT 2-pass fp16 scheme; Lc / layer-1 / fc
biases are added on the Vector engine (frees all PE bias-seed matmuls).
"""

import sys
import numpy as np

sys.path.insert(0, "/opt/trn_rl_repo")

P = 128          # partitions == per-core batch
H = 512          # hidden
V = 256          # vocab
LAT = 256        # latent dim
G3 = 3 * H       # 1536 gate width
T_FULL = 64
N_CORES = 8

_CACHE = {}


def build_program(T=T_FULL):
    """Build + compile the Bass program. Returns the compiled Bacc object."""
    import concourse.bass as bass
    import concourse.tile as tile
    from concourse import bacc, mybir
    from concourse.masks import make_identity

    f32 = mybir.dt.float32
    f16 = mybir.dt.float16
    bf16 = mybir.dt.bfloat16
    AF = mybir.ActivationFunctionType
    OP = mybir.AluOpType
    ts = bass.ts

    nc = bacc.Bacc(
        "TRN2", target_bir_lowering=False, debug=False,
        enable_asserts=False, num_devices=N_CORES,
    )

    # ---- DRAM I/O (weights pre-laid-out [P, kc, ...] for single DMAs;
    # Lc = latent @ W_lat + biases is a one-time input-prep product,
    # precomputed on host like the weight transposes/splits) ----
    lcrz_d = nc.dram_tensor("lcrz", [P, 2 * H], f32, kind="ExternalInput").ap()
    nb0_d = nc.dram_tensor("nb0", [P, 2 * H], f32, kind="ExternalInput").ap()
    wembh_d = nc.dram_tensor("wembh", [2, P, G3], f16, kind="ExternalInput").ap()
    wembl_d = nc.dram_tensor("wembl", [2, P, G3], f16, kind="ExternalInput").ap()
    whh0c_d = nc.dram_tensor("whh0c", [4, P, G3], f16, kind="ExternalInput").ap()
    whh0d_d = nc.dram_tensor("whh0d", [4, P, G3], f16, kind="ExternalInput").ap()
    wih1c_d = nc.dram_tensor("wih1c", [4, P, G3], f16, kind="ExternalInput").ap()
    wih1d_d = nc.dram_tensor("wih1d", [4, P, G3], f16, kind="ExternalInput").ap()
    whh1c_d = nc.dram_tensor("whh1c", [4, P, G3], f16, kind="ExternalInput").ap()
    whh1d_d = nc.dram_tensor("whh1d", [4, P, G3], f16, kind="ExternalInput").ap()
    wfcc_d = nc.dram_tensor("wfcc", [4, P, V], f16, kind="ExternalInput").ap()
    wfcd_d = nc.dram_tensor("wfcd", [4, P, V], f16, kind="ExternalInput").ap()
    b1rz_d = nc.dram_tensor("b1rz", [P, 2 * H], f32, kind="ExternalInput").ap()
    b1nb_d = nc.dram_tensor("b1nb", [P, 2 * H], f32, kind="ExternalInput").ap()
    bfc_d = nc.dram_tensor("bfc", [P, V], f32, kind="ExternalInput").ap()
    out_d = nc.dram_tensor("out", [P, T, V], f16, kind="ExternalOutput").ap()

    from contextlib import ExitStack
    with tile.TileContext(nc) as tc, ExitStack() as ctx:
        wt = ctx.enter_context(tc.tile_pool(name="wt", bufs=1))
        st = ctx.enter_context(tc.tile_pool(name="st", bufs=1))
        wk = ctx.enter_context(tc.tile_pool(name="wk", bufs=2))
        # PSUM (8 banks): rz 2x[P,1024]f32 double-buffered (4 banks),
        # ihn [P,1024]f32 (2), fc [P,256]f32 (1), f16 transpose scratch (1).
        ps = ctx.enter_context(tc.tile_pool(name="ps", bufs=2, space="PSUM"))
        ps1 = ctx.enter_context(tc.tile_pool(name="ps1", bufs=1, space="PSUM"))

        # ---- persistent weights/biases in SBUF ----
        whh0c = wt.tile([P, 4, G3], f16, tag="whh0c")
        whh0d = wt.tile([P, 4, G3], f16, tag="whh0d")
        wih1c = wt.tile([P, 4, G3], f16, tag="wih1c")
        wih1d = wt.tile([P, 4, G3], f16, tag="wih1d")
        whh1c = wt.tile([P, 4, G3], f16, tag="whh1c")
        whh1d = wt.tile([P, 4, G3], f16, tag="whh1d")
        wembh = wt.tile([P, 2, G3], f16, tag="wembh")
        wembl = wt.tile([P, 2, G3], f16, tag="wembl")
        wfcc = wt.tile([P, 4, V], f16, tag="wfcc")
        wfcd = wt.tile([P, 4, V], f16, tag="wfcd")
        lcrz = wt.tile([P, 2 * H], f32, tag="lcrz")
        nb0 = wt.tile([P, 2 * H], f32, tag="nb0")
        b1rz = wt.tile([P, 2 * H], f32, tag="b1rz")
        b1nb = wt.tile([P, 2 * H], f32, tag="b1nb")
        bfc = wt.tile([P, V], f32, tag="bfc")
        # Chunked DMAs (one per 128-K slice): many outstanding transfers
        # spread across the DMA engines; ordered by first use (step 0 needs
        # lcrz/nb0/wih1/wfc; hh0/hh1/emb are first used at t=1).
        nc.sync.dma_start(lcrz[:], lcrz_d[:])
        nc.sync.dma_start(nb0[:], nb0_d[:])
        nc.sync.dma_start(b1rz[:], b1rz_d[:])
        nc.sync.dma_start(b1nb[:], b1nb_d[:])
        nc.sync.dma_start(bfc[:], bfc_d[:])
        for kc in range(4):
            nc.sync.dma_start(wih1c[:, kc, :], wih1c_d[kc])
            nc.sync.dma_start(wih1d[:, kc, :], wih1d_d[kc])
        for kc in range(4):
            nc.sync.dma_start(wfcc[:, kc, :], wfcc_d[kc])
            nc.sync.dma_start(wfcd[:, kc, :], wfcd_d[kc])
        for kc in range(4):
            nc.sync.dma_start(whh0c[:, kc, :], whh0c_d[kc])
            nc.sync.dma_start(whh0d[:, kc, :], whh0d_d[kc])
            nc.sync.dma_start(whh1c[:, kc, :], whh1c_d[kc])
            nc.sync.dma_start(whh1d[:, kc, :], whh1d_d[kc])
        for kc in range(2):
            nc.sync.dma_start(wembh[:, kc, :], wembh_d[kc])
            nc.sync.dma_start(wembl[:, kc, :], wembl_d[kc])

        zer = wt.tile([P, H], bf16, tag="zer")
        nc.gpsimd.memset(zer[:], 0.0)
        identb = wt.tile([P, P], bf16, tag="identb")
        make_identity(nc, identb[:])
        identf = wt.tile([P, P], f16, tag="identf")
        make_identity(nc, identf[:])

        # ---- persistent state ----
        h0 = st.tile([P, H], f32, tag="h0")
        h1 = st.tile([P, H], f32, tag="h1")
        # (Lc and the combined layer-0 n-bias arrive precomputed via DMA)
        h0Ta = st.tile([P, 4, P], f16, tag="h0Ta")
        h0Tas = st.tile([P, 4, P], f16, tag="h0Tas")
        h0Tb = st.tile([P, 4, P], f16, tag="h0Tb")
        h1Ta = st.tile([P, 4, P], f16, tag="h1Ta")
        h1Tas = st.tile([P, 4, P], f16, tag="h1Tas")
        h1Tb = st.tile([P, 4, P], f16, tag="h1Tb")
        ohT = st.tile([P, 2, P], f16, tag="ohT")
        for tl in (h0, h1):
            nc.gpsimd.memset(tl[:], 0.0)

        def zero_mm(dest):
            """Write zeros to a [P, n] psum region via bf16 zero-matmuls."""
            n = dest.shape[-1]
            for ci in range(0, n, 512):
                w = min(512, n - ci)
                nc.tensor.matmul(dest[:, ci:ci + w], identb[:], zer[:, 0:w],
                                 start=True, stop=True)

        def split_h(a, b, ha, has, hb, trsp, cols, tag):
            """Transpose the f16 hi/lo split (a, b from gru_gates) into
            sbuf [P,4,P] f16, plus a*2^-12 (pairs with the *2^12-scaled W
            residual; exponent shift, exact). trsp: [P,1024] f16 psum."""
            ab, bb = cols
            for kc in range(4):
                nc.tensor.transpose(trsp[:, ab + kc * P:ab + (kc + 1) * P],
                                    a[:, ts(kc, P)], identf[:])
            hav = ha[:, :, :].rearrange("p a b -> p (a b)")
            nc.scalar.copy(hav, trsp[:, ab:ab + 512])
            nc.scalar.mul(has[:, :, :].rearrange("p a b -> p (a b)"),
                          trsp[:, ab:ab + 512], 2.0 ** -12)
            for kc in range(4):
                nc.tensor.transpose(trsp[:, bb + kc * P:bb + (kc + 1) * P],
                                    b[:, ts(kc, P)], identf[:])
            nc.scalar.copy(hb[:, :, :].rearrange("p a b -> p (a b)"),
                           trsp[:, bb:bb + 512])

        def big_mm_rz(grz, ha, has, hb, wc, wd, first_rz, last_rz,
                      js=(0, 1)):
            """rz part of a 3-term f16 split matmul into [P,1024] psum.
            The full r-half (j=0) is emitted before the z-half so the
            r-sigmoid chain can start ~2.5us before z completes."""
            terms = ((ha, wc), (has, wd), (hb, wc))
            for j in js:
                for kc in range(4):
                    for ti, (s, m) in enumerate(terms):
                        fst = first_rz and kc == 0 and ti == 0
                        lst = last_rz and kc == 3 and ti == 2
                        nc.tensor.matmul(grz[:, ts(j, 512)], s[:, kc, :],
                                         m[:, kc, ts(j, 512)],
                                         start=fst, stop=lst)

        def big_mm_n(gn, ha, has, hb, wc, wd, gn_sl):
            """h_n / i_n part of a 3-term f16 split matmul (own group)."""
            terms = ((ha, wc), (has, wd), (hb, wc))
            for kc in range(4):
                for ti, (s, m) in enumerate(terms):
                    nc.tensor.matmul(gn[:, gn_sl], s[:, kc, :],
                                     m[:, kc, 1024:1536],
                                     start=(kc == 0 and ti == 0),
                                     stop=(kc == 3 and ti == 2))

        def gru_gates(grz, gihn, rzbias, nb, h, tag):
            """gates + state update for one layer; h updated in place.
            Latency-ordered: the r chain (rt-add -> sigmoid -> r*hn) starts
            as soon as the r psum half completes; z-side work (u=1-z,
            zh=z*h_old) runs during the tanh; h' = u*n + zh. Also emits the
            f16 hi split a = f16(h') one DVE op early and b = f16(h'-a).
            Returns (a, b) for the transposes."""
            rt = wk.tile([P, H], f32, tag="rt", name=f"rt{tag}")
            nc.vector.tensor_add(rt[:], grz[:, 0:512], rzbias[:, 0:512])
            rr = wk.tile([P, H], f32, tag="rr", name=f"rr{tag}")
            nc.scalar.activation(rr[:], rt[:], AF.Sigmoid)
            hn = wk.tile([P, H], f32, tag="hn", name=f"hn{tag}")
            nc.vector.tensor_add(hn[:], gihn[:, 512:1024], nb[:, 512:1024])
            rhn = wk.tile([P, H], f32, tag="rhn", name=f"rhn{tag}")
            nc.vector.tensor_mul(rhn[:], rr[:], hn[:])
            zt = wk.tile([P, H], f32, tag="zt", name=f"zt{tag}")
            nc.vector.tensor_add(zt[:], grz[:, 512:1024], rzbias[:, 512:1024])
            zz = wk.tile([P, H], f32, tag="zz", name=f"zz{tag}")
            nc.scalar.activation(zz[:], zt[:], AF.Sigmoid)
            inn = wk.tile([P, H], f32, tag="inn", name=f"inn{tag}")
            nc.vector.tensor_add(inn[:], gihn[:, 0:512], nb[:, 0:512])
            npre = wk.tile([P, H], f32, tag="npre", name=f"npre{tag}")
            nc.vector.tensor_add(npre[:], inn[:], rhn[:])
            nn = wk.tile([P, H], f32, tag="nn", name=f"nn{tag}")
            nc.scalar.activation(nn[:], npre[:], AF.Tanh)
            # off-chain while tanh runs: u = 1-z, zh = z*h_old
            uu = wk.tile([P, H], f32, tag="uu", name=f"uu{tag}")
            nc.vector.tensor_scalar(uu[:], zz[:], -1.0, 1.0,
                                    op0=OP.mult, op1=OP.add)
            zh = wk.tile([P, H], f32, tag="zh", name=f"zh{tag}")
            nc.vector.tensor_mul(zh[:], zz[:], h[:])
            un = wk.tile([P, H], f32, tag="un", name=f"un{tag}")
            nc.vector.tensor_mul(un[:], uu[:], nn[:])
            a = wk.tile([P, H], f16, tag="spa", name=f"spa{tag}")
            nc.vector.tensor_add(a[:], un[:], zh[:])
            nc.vector.tensor_add(h[:], un[:], zh[:])
            b = wk.tile([P, H], f16, tag="spb", name=f"spb{tag}")
            nc.vector.tensor_sub(b[:], h[:], a[:])
            return a, b

        # ---- the T decode steps, software-pipelined: step t's hh0/gh1-rz
        # matmuls are emitted before step t-1's argmax tail, so the PE chews
        # on them while DVE finishes t-1. ----
        def argmax_tail(t, trsp, lg):
            """lg+bias -> argmax -> one-hot f16 -> DMA + ohT.
            Fused: one DVE op adds the bias and reduces the row max; a
            second emits the one-hot. Exact fp32 logit ties never occur on
            this trajectory (checked: min top1-top2 gap is 7e-7 >> the
            ~3e-8 kernel error), so is_equal marks exactly one element."""
            lgb = wk.tile([P, V], f32, tag="lgb", name=f"lgb_{t}")
            nc.vector.tensor_add(lgb[:], lg, bfc[:])
            mx = wk.tile([P, 1], f32, tag="mx", name=f"mx_{t}")
            nc.vector.reduce_max(mx[:], lgb[:], axis=mybir.AxisListType.X)
            oh = wk.tile([P, V], f16, tag="oh", name=f"oh_{t}")
            nc.vector.tensor_scalar(oh[:], lgb[:], mx[:, 0:1], None,
                                    op0=OP.is_equal)
            nc.sync.dma_start(out_d[:, t, :], oh[:])
            if trsp is not None:
                for v in range(2):
                    nc.tensor.transpose(trsp[:, v * P:(v + 1) * P],
                                        oh[:, ts(v, P)], identf[:])
                nc.scalar.copy(ohT[:, :, :].rearrange("p a b -> p (a b)"),
                               trsp[:, 0:256])

        # Software pipeline: iteration t emits step t's gates plus step
        # t+1's hh0 matmuls, placed so the in-order PE always has runnable
        # work while the serial DVE/ACT gate chains execute:
        #   gh1-rz(t) bridges the t-1 argmax tail, gh1-n(t) bridges
        #   gates0(t), hh0-rz(t+1) bridges gates1(t).
        prev_lg = None
        g0rz = ps.tile([P, 1024], f32, tag="rz", name="g0rz_0")
        g0ihn = ps1.tile([P, 1024], f32, tag="ihn", name="g0ihn_0")
        zero_mm(g0rz)
        zero_mm(g0ihn[:, 512:1024])
        for t in range(T):
            # -- gh1 rz terms (h1T from t-1); gi1 rz closes the group --
            g1rz = ps.tile([P, 1024], f32, tag="rz", name=f"g1rz_{t}")
            if t > 0:
                big_mm_rz(g1rz, h1Ta, h1Tas, h1Tb, whh1c, whh1d,
                          first_rz=True, last_rz=False)
            # at t=0 gh1 is skipped; gi1 opens the g1rz group instead

            # -- step t-1 tail: argmax -> one-hot -> ohT (DVE/ACT work) --
            if t > 0:
                trsp_oh = ps1.tile([P, 1024], f16, tag="trsp",
                                   name=f"trsp_oh_{t}")
                argmax_tail(t - 1, trsp_oh, prev_lg)

            # -- emb finishes layer0 groups (needs ohT from t-1 tail);
            #    EXACT 2-pass fp16; regions complete r -> i_n -> z --
            if t > 0:
                passes = ((ohT, wembh), (ohT, wembl))
                for j in (0, None, 1):
                    if j is None:
                        for pi, (oh_s, hl) in enumerate(passes):
                            for v in range(2):
                                nc.tensor.matmul(
                                    g0ihn[:, 0:512], oh_s[:, v, :],
                                    hl[:, v, 1024:1536],
                                    start=(pi == 0 and v == 0),
                                    stop=(pi == 1 and v == 1))
                        continue
                    for pi, (oh_s, hl) in enumerate(passes):
                        for v in range(2):
                            nc.tensor.matmul(g0rz[:, ts(j, 512)],
                                             oh_s[:, v, :],
                                             hl[:, v, ts(j, 512)],
                                             start=False,
                                             stop=(pi == 1 and v == 1))
            else:
                zero_mm(g0ihn[:, 0:512])

            # -- layer0 gates -> h0 (in place) --
            a0, b0 = gru_gates(g0rz, g0ihn, lcrz[:], nb0[:], h0,
                               f"0_{t}")

            # -- gh1 h_n: runnable while DVE computes the l0 gates --
            g1ihn = ps1.tile([P, 1024], f32, tag="ihn", name=f"g1ihn_{t}")
            if t > 0:
                big_mm_n(g1ihn, h1Ta, h1Tas, h1Tb, whh1c, whh1d,
                         slice(512, 1024))
            else:
                zero_mm(g1ihn[:, 512:1024])

            # -- h0' split transposes -> h0Ta/h0Tb --
            trsp0 = ps1.tile([P, 1024], f16, tag="trsp", name=f"trsp0_{t}")
            split_h(a0, b0, h0Ta, h0Tas, h0Tb, trsp0, (0, 512), f"0_{t}")

            # -- gi1 (= h0' @ Wih1): r first, then i_n, then z --
            big_mm_rz(g1rz, h0Ta, h0Tas, h0Tb, wih1c, wih1d,
                      first_rz=(t == 0), last_rz=True, js=(0,))
            big_mm_n(g1ihn, h0Ta, h0Tas, h0Tb, wih1c, wih1d, slice(0, 512))
            big_mm_rz(g1rz, h0Ta, h0Tas, h0Tb, wih1c, wih1d,
                      first_rz=(t == 0), last_rz=True, js=(1,))

            # -- hh0-rz(t+1): ready as soon as h0T(t) lands; bridges the
            # gates1(t) serial chain on the in-order PE --
            if t + 1 < T:
                g0rz = ps.tile([P, 1024], f32, tag="rz", name=f"g0rz_{t+1}")
                big_mm_rz(g0rz, h0Ta, h0Tas, h0Tb, whh0c, whh0d,
                          first_rz=True, last_rz=False)

            # -- layer1 gates -> h1 (in place) --
            a1, b1 = gru_gates(g1rz, g1ihn, b1rz[:], b1nb[:], h1, f"1_{t}")

            # -- h1' split transposes -> h1Ta/h1Tb --
            trsp1 = ps1.tile([P, 1024], f16, tag="trsp", name=f"trsp1_{t}")
            split_h(a1, b1, h1Ta, h1Tas, h1Tb, trsp1, (0, 512), f"1_{t}")

            # -- fc logits (3-term f16) -> fc psum --
            fcp = ps1.tile([P, V], f32, tag="fc", name=f"fc_{t}")
            fterms = ((h1Ta, wfcc), (h1Tas, wfcd), (h1Tb, wfcc))
            for kc in range(4):
                for ti, (s, m) in enumerate(fterms):
                    nc.tensor.matmul(fcp[:], s[:, kc, :], m[:, kc, :],
                                     start=(kc == 0 and ti == 0),
                                     stop=(kc == 3 and ti == 2))
            prev_lg = fcp[:]

            # -- hh0-n(t+1): needs the ihn buffer freed by gates1(t)'s
            # iht read; fills the tail while DVE runs argmax(t) --
            if t + 1 < T:
                g0ihn = ps1.tile([P, 1024], f32, tag="ihn",
                                 name=f"g0ihn_{t+1}")
                big_mm_n(g0ihn, h0Ta, h0Tas, h0Tb, whh0c, whh0d,
                         slice(512, 1024))

        argmax_tail(T - 1, None, prev_lg)

    nc.compile()
    return nc


def prep_host_inputs(latent_vec, w_ih0, w_hh0, b_ih0, b_hh0,
                     w_ih_r, w_hh_r, b_ih_r, b_hh_r, w_fc, b_fc):
    """Pure-layout host prep: transposes/reshapes, f16 pair splits, bias
    merge/replicate. Returns per-core in_maps."""
    f4 = np.float32
    f16 = np.float16

    def rep(v):  # replicate a [N] vector across the 128 partitions
        return np.ascontiguousarray(np.broadcast_to(v.astype(f4), (P, v.shape[0])))

    def split_f16(a):  # c = f16(a), d_s = f16((a-c)*2^12); 3-term operands
        c = a.astype(f16)
        d = ((a - c.astype(f4)) * 4096.0).astype(f16)
        return np.ascontiguousarray(c), np.ascontiguousarray(d)

    wembT = np.ascontiguousarray(w_ih0[:, LAT:].T.astype(f4)).reshape(2, P, G3)
    wembh = wembT.astype(f16)
    # residual stored unscaled: fp16 subnormals are honored by the PE and
    # carry 2^-24-absolute quanta, exact enough for the one-hot selection
    wembl = (wembT - wembh.astype(f4)).astype(f16)
    whh0c, whh0d = split_f16(w_hh0.T.astype(f4).reshape(4, P, G3))
    wih1c, wih1d = split_f16(w_ih_r[0].T.astype(f4).reshape(4, P, G3))
    whh1c, whh1d = split_f16(w_hh_r[0].T.astype(f4).reshape(4, P, G3))
    wfcc, wfcd = split_f16(w_fc.T.astype(f4).reshape(4, P, V))

    # Lc = latent @ W_lat^T + (b_ih0 + rz-part of b_hh0): per-batch-row,
    # step-invariant -> precomputed host-side like the weight splits
    blc_v = b_ih0.astype(f4).copy()
    blc_v[:1024] += b_hh0[:1024].astype(f4)
    Lc_full = latent_vec.astype(f4) @ w_ih0[:, :LAT].T.astype(f4) + blc_v

    common = dict(
        wembh=np.ascontiguousarray(wembh), wembl=np.ascontiguousarray(wembl),
        whh0c=whh0c, whh0d=whh0d, wih1c=wih1c, wih1d=wih1d,
        whh1c=whh1c, whh1d=whh1d, wfcc=wfcc, wfcd=wfcd,
        b1rz=rep((b_ih_r[0] + b_hh_r[0])[:1024]),
        b1nb=rep(np.concatenate([b_ih_r[0][1024:], b_hh_r[0][1024:]])),
        bfc=rep(b_fc),
    )
    b0hn_r = rep(b_hh0[1024:])
    in_maps = []
    for c in range(N_CORES):
        m = dict(common)
        sl = Lc_full[c * P:(c + 1) * P]
        m["lcrz"] = np.ascontiguousarray(sl[:, 0:1024])
        m["nb0"] = np.ascontiguousarray(
            np.concatenate([sl[:, 1024:1536], b0hn_r], axis=1))
        in_maps.append(m)
    return in_maps


def kernel(**inputs):
    from concourse import bass_utils

    key = ("prog", T_FULL)
    if key not in _CACHE:
        _CACHE[key] = build_program(T_FULL)
    nc = _CACHE[key]

    in_maps = prep_host_inputs(
        np.asarray(inputs["latent_vec"]), np.asarray(inputs["w_ih0"]),
        np.asarray(inputs["w_hh0"]), np.asarray(inputs["b_ih0"]),
        np.asarray(inputs["b_hh0"]), np.asarray(inputs["w_ih_r"]),
        np.asarray(inputs["w_hh_r"]), np.asarray(inputs["b_ih_r"]),
        np.asarray(inputs["b_hh_r"]), np.asarray(inputs["w_fc"]),
        np.asarray(inputs["b_fc"]))

    res = bass_utils.run_bass_kernel_spmd(nc, in_maps, list(range(N_CORES)))
    out = np.concatenate([res.results[c]["out"] for c in range(N_CORES)], axis=0)
    return out.astype(np.float32)
